# revision 34
# baseline (speedup 1.0000x reference)
"""Self-contained Trainium2 Bass kernel for BertSelfAttention (relative_key_query).

kernel(**inputs) takes FULL unsharded inputs (as in setup_inputs()) and returns
the FULL (8, 1024, 1024) float32 output. Internally: data-parallel over the
batch dimension, one batch per NeuronCore across 8 cores, via
concourse run_bass_kernel_spmd.

Math (per batch b): qkv = hs @ W_qkv + b; per-head q,k,v (dh=64);
scores = (q k^T + q.pe^T-window + k.pe-window)/8 + mask; softmax; ctx = p v.
The relative-position terms are computed as banded "windows" against the
2047x64 distance table, stored to DRAM in fp8 (scaled by 8), and read back
with a skewed access pattern that materializes the diagonal bands densely.
Dense matmuls run in bf16; window/score-injection matmuls run in fp8.
"""

import os
import numpy as np


import concourse.bacc as bacc
import concourse.mybir as mybir
import concourse.tile as tile

f32 = mybir.dt.float32
bf16 = mybir.dt.bfloat16
fp8 = mybir.dt.float8e4

S = 1024
D = 1024
H = 16
DH = 64
NT = 8
WIN = 1152
NPAIR = 8


def host_prep(hidden_states, attention_mask, W_qkv, b_qkv, dist_emb):
    import ml_dtypes
    bfl = ml_dtypes.bfloat16
    e4 = ml_dtypes.float8_e4m3fn

    B = hidden_states.shape[0]
    W = np.asarray(W_qkv, dtype=np.float32)
    b = np.asarray(b_qkv, dtype=np.float32)
    T = np.asarray(dist_emb, dtype=np.float32)

    qcols = np.zeros((8, 128), dtype=np.int64)
    kcols = np.zeros((8, 128), dtype=np.int64)
    for t in range(8):
        for j in range(128):
            h = 2 * t + (j >= 64)
            d = j % 64
            qcols[t, j] = h * 192 + d
            kcols[t, j] = h * 192 + 64 + d
    qk_idx = np.concatenate([qcols.reshape(-1), kcols.reshape(-1)])
    WQK = np.ascontiguousarray(W[:, qk_idx]).astype(bfl)
    bQK = np.ascontiguousarray(b[qk_idx].reshape(16, 128).T)
    vidx = np.array([h * 192 + 128 + d for h in range(H) for d in range(DH)])
    WV = np.ascontiguousarray(W[:, vidx]).astype(bfl)
    bV = np.ascontiguousarray(b[vidx].reshape(1, 1024)).astype(bfl)

    T2 = np.zeros((128, 2048), dtype=np.float32)
    T2[0:64, 0:2047] = T.T
    T2[64:128, 0:2047] = T.T
    T2R = np.zeros((128, 2048), dtype=np.float32)
    T2R[0:64, 0:2047] = T.T[:, ::-1]
    T2R[64:128, 0:2047] = T.T[:, ::-1]
    T2 = T2.astype(bfl)
    T2R = T2R.astype(bfl)

    ones_r = np.ones((1, 128), dtype=np.float32).astype(bfl)
    # identity used to inject fp8 windows into the score PSUM; its value
    # 0.125 cancels the x8 scaling applied when the windows were stored.
    id8_h = (0.125 * np.eye(128, dtype=np.float32)).astype(e4)
    idb_h = np.eye(128, dtype=np.float32).astype(bfl)
    idf_h = np.eye(128, dtype=np.float32)

    mask = np.asarray(attention_mask, dtype=np.float32).reshape(B, S)
    in_maps = []
    for bi in range(B):
        mhat = np.ascontiguousarray(mask[bi].reshape(8, 128).T)
        in_maps.append({
            "hs": np.ascontiguousarray(hidden_states[bi], dtype=np.float32),
            "wqk": WQK, "bqk": bQK, "wv": WV, "bv": bV,
            "t2": T2, "t2r": T2R, "ones_r": ones_r, "mhat": mhat,
            "id8_h": id8_h, "idb_h": idb_h, "idf_h": idf_h,
        })
    return in_maps


def build_program(npair=NPAIR):
    nc = bacc.Bacc()
    hs_d = nc.declare_dram_parameter("hs", [S, D], f32, isOutput=False)
    wqk_d = nc.declare_dram_parameter("wqk", [D, 2048], bf16, isOutput=False)
    bqk_d = nc.declare_dram_parameter("bqk", [128, 16], f32, isOutput=False)
    wv_d = nc.declare_dram_parameter("wv", [D, 1024], bf16, isOutput=False)
    bv_d = nc.declare_dram_parameter("bv", [1, 1024], bf16, isOutput=False)
    t2_d = nc.declare_dram_parameter("t2", [128, 2048], bf16, isOutput=False)
    t2r_d = nc.declare_dram_parameter("t2r", [128, 2048], bf16, isOutput=False)
    ones_d = nc.declare_dram_parameter("ones_r", [1, 128], bf16, isOutput=False)
    mhat_d = nc.declare_dram_parameter("mhat", [128, 8], f32, isOutput=False)
    id8_d = nc.declare_dram_parameter("id8_h", [128, 128], fp8, isOutput=False)
    idb_d = nc.declare_dram_parameter("idb_h", [128, 128], bf16, isOutput=False)
    idf_d = nc.declare_dram_parameter("idf_h", [128, 128], f32, isOutput=False)
    out_d = nc.declare_dram_parameter("out", [S, D], f32, isOutput=True)

    Exp = mybir.ActivationFunctionType.Exp
    Ident = mybir.ActivationFunctionType.Identity

    with tile.TileContext(nc) as tc:
        with tc.tile_pool(name="const", bufs=1) as cpool:
            t2_sb = cpool.tile([128, 2048], bf16, tag="t2", name="t2")
            nc.sync.dma_start(t2_sb[:], t2_d.ap())
            t2r_sb = cpool.tile([128, 2048], bf16, tag="t2r", name="t2r")
            nc.sync.dma_start(t2r_sb[:], t2r_d.ap())
            bqk_sb = cpool.tile([128, 16], f32, tag="bqk", name="bqk")
            nc.sync.dma_start(bqk_sb[:], bqk_d.ap())
            bv_sb = cpool.tile([1, 1024], bf16, tag="bv", name="bv")
            nc.sync.dma_start(bv_sb[:], bv_d.ap())
            ones_sb = cpool.tile([1, 128], bf16, tag="ones", name="ones")
            nc.sync.dma_start(ones_sb[:], ones_d.ap())
            mhat_sb = cpool.tile([128, 8], f32, tag="mh", name="mh")
            nc.sync.dma_start(mhat_sb[:], mhat_d.ap())
            id8 = cpool.tile([128, 128], fp8, tag="id8", name="id8")
            nc.sync.dma_start(id8[:], id8_d.ap())
            idb = cpool.tile([128, 128], bf16, tag="idb", name="idb")
            nc.sync.dma_start(idb[:], idb_d.ap())
            idf = cpool.tile([128, 128], f32, tag="idf", name="idf")
            nc.sync.dma_start(idf[:], idf_d.ap())
            ones16 = cpool.tile([128, 16], bf16, tag="o16", name="o16")
            nc.vector.memset(ones16[:], 1.0)

            # resident weights (bf16); DMAs are emitted after phase 1 so the
            # hs loads that gate the first transposes go out first.
            wqk_sb = [cpool.tile([128, 2048], bf16, tag=f"wqk{i}", name=f"wqk{i}")
                      for i in range(NT)]
            wv_sb = [cpool.tile([128, 1024], bf16, tag=f"wv{i}", name=f"wv{i}")
                     for i in range(NT)]

            hsT = [cpool.tile([128, 1024], bf16, tag=f"hsT{i}", name=f"hsT{i}") for i in range(NT)]
            vh = [cpool.tile([128, 1040], bf16, tag=f"vh{t}", name=f"vh{t}") for t in range(NT)]
            outacc = [cpool.tile([128, 1024], f32, tag=f"oa{i}", name=f"oa{i}") for i in range(NT)]
            for L in range(NT):
                nc.gpsimd.memset(outacc[L][:], 0.0)

            # ---- Phase 1: hs -> bf16 -> hsT via PE transpose ----
            with tc.tile_pool(name="p1", bufs=2) as p1, \
                 tc.tile_pool(name="p1ps", bufs=2, space="PSUM") as p1ps:
                for lt in range(NT):
                    hstile = p1.tile([128, 1024], f32, tag="hs", name="hs")
                    nc.sync.dma_start(hstile[:], hs_d.ap()[128 * lt:128 * (lt + 1), :])
                    hsb = p1.tile([128, 1024], bf16, tag="hsb", name="hsb")
                    nc.gpsimd.tensor_copy(hsb[:], hstile[:])
                    for ig in range(2):
                        ps = p1ps.tile([128, 512], bf16, tag="tp", name="tp")
                        for j in range(4):
                            it = 4 * ig + j
                            nc.tensor.matmul(
                                ps[:, 128 * j:128 * (j + 1)],
                                hsb[:, 128 * it:128 * (it + 1)], idb[:],
                                is_transpose=True,
                                start=(j == 0), stop=(j == 3),
                                skip_group_check=True)
                        for j in range(4):
                            it = 4 * ig + j
                            nc.scalar.copy(
                                hsT[it][:, 128 * lt:128 * (lt + 1)],
                                ps[:, 128 * j:128 * (j + 1)])
                    if lt == 0:
                        for i in range(NT):
                            nc.sync.dma_start(
                                wv_sb[i][:], wv_d.ap()[128 * i:128 * (i + 1), :])
                    if lt == 1:
                        for i in range(NT):
                            nc.sync.dma_start(
                                wqk_sb[i][:],
                                wqk_d.ap()[128 * i:128 * (i + 1), :])

            # ---- Phase 2: v-hat tiles (+ones col, +bias) ----
            with tc.tile_pool(name="p2ps", bufs=3, space="PSUM") as p2ps:
                for tau in range(NT):
                    lt = tau
                    psv = p2ps.tile([128, 1024], f32, tag="vps", name="vps")
                    for half in range(2):
                        sl = slice(512 * half, 512 * (half + 1))
                        nc.tensor.matmul(psv[:, sl], ones_sb[:], bv_sb[:, sl],
                                         start=True, stop=False,
                                         skip_group_check=True)
                    for it in range(NT):
                        for half in range(2):
                            sl = slice(512 * half, 512 * (half + 1))
                            nc.tensor.matmul(psv[:, sl],
                                             hsT[it][:, 128 * lt:128 * (lt + 1)],
                                             wv_sb[it][:, sl],
                                             start=False, stop=(it == NT - 1),
                                             skip_group_check=True)
                    out_ap = vh[tau][:].__replace__(
                        ap=[[1040, 128], [65, 16], [1, 64]], offset=0)
                    in_ap = psv[:].__replace__(
                        ap=[[1024, 128], [64, 16], [1, 64]], offset=0)
                    nc.scalar.copy(out_ap, in_ap)
                    ones_ap = vh[tau][:].__replace__(
                        ap=[[1040, 128], [65, 16]], offset=64)
                    nc.scalar.copy(ones_ap, ones16[:])

            # ---- Phase 3: per head-pair ----
            with tc.tile_pool(name="qk", bufs=2) as qkpool, \
                 tc.tile_pool(name="winsb", bufs=4) as winsb, \
                 tc.tile_pool(name="sblk", bufs=4) as sblkpool, \
                 tc.tile_pool(name="s1blk", bufs=16) as s1pool, \
                 tc.tile_pool(name="probs", bufs=6) as prpool, \
                 tc.tile_pool(name="ctxsb", bufs=2) as ctxsb, \
                 tc.tile_pool(name="osmall", bufs=4) as osmall, \
                 tc.tile_pool(name="dram", bufs=36, space="DRAM") as dpool, \
                 tc.tile_pool(name="small", bufs=1, space="PSUM") as smallps, \
                 tc.tile_pool(name="winps", bufs=3, space="PSUM") as winps, \
                 tc.tile_pool(name="scps", bufs=2, space="PSUM") as scps, \
                 tc.tile_pool(name="ctxps", bufs=1, space="PSUM") as ctxps:
                for P in range(npair):
                    # -- 3a: q^T, k^T (bf16) --
                    qT = qkpool.tile([128, 1024], bf16, tag="qT", name="qT")
                    kT = qkpool.tile([128, 1024], bf16, tag="kT", name="kT")
                    for dst, ct in ((qT, P), (kT, 8 + P)):
                        for half in range(2):
                            sl = slice(512 * half, 512 * (half + 1))
                            ps = smallps.tile([128, 512], f32, tag="small", name="ps")
                            for it in range(NT):
                                nc.tensor.matmul(
                                    ps[:],
                                    wqk_sb[it][:, 128 * ct:128 * (ct + 1)],
                                    hsT[it][:, sl],
                                    start=(it == 0), stop=(it == NT - 1),
                                    skip_group_check=True)
                            nc.scalar.activation(dst[:, sl], ps[:], Ident,
                                                 bias=bqk_sb[:, ct:ct + 1],
                                                 scale=1.0)

                    # -- 3b: windows -> fp8 (x8) -> DRAM, plus skewed reads --
                    # casts are spread over vector/gpsimd/scalar so the PE
                    # window matmuls are not cast-throughput-bound.
                    qd_dram = [[], []]
                    kd_dram = [[], []]
                    s1tiles = [[], []]
                    for hh in range(2):
                        rs = slice(64 * hh, 64 * (hh + 1))
                        for t in range(NT):
                            base = 896 - 128 * t
                            # q-side: fp8 (x8) — feeds the weight-load-bound
                            # transpose injects, which need 1-byte weights.
                            w8q = winsb.tile([128, WIN], fp8, tag="w8", name="w8")
                            for c3 in range(3):
                                wps = winps.tile([128, 384], f32, tag="win", name="wps")
                                nc.tensor.matmul(
                                    wps[:], qT[rs, 128 * t:128 * (t + 1)],
                                    t2r_sb[rs, base + 384 * c3:base + 384 * (c3 + 1)],
                                    start=True, stop=True, skip_group_check=True)
                                nc.vector.tensor_scalar_mul(
                                    w8q[:, 384 * c3:384 * (c3 + 1)], wps[:], 8.0)

                            dq = dpool.tile([128, WIN], fp8, tag="dq", name="dq")
                            nc.sync.dma_start(dq[:], w8q[:])
                            qd_dram[hh].append(dq)
                            blk = s1pool.tile([128, 1024], fp8, tag="s1", name="s1")
                            nc.sync.dma_start(blk[:], dq[:].__replace__(
                                ap=[[1151, 128], [1, 1024]], offset=127))
                            s1tiles[hh].append(blk)

                            # k-side: fp8 (x8), injected as moving data
                            w8k = winsb.tile([128, WIN], fp8, tag="w8k", name="w8k")
                            for c3 in range(3):
                                wps = winps.tile([128, 384], f32, tag="win", name="wps")
                                nc.tensor.matmul(
                                    wps[:], kT[rs, 128 * t:128 * (t + 1)],
                                    t2_sb[rs, base + 384 * c3:base + 384 * (c3 + 1)],
                                    start=True, stop=True, skip_group_check=True)
                                nc.scalar.mul(w8k[:, 384 * c3:384 * (c3 + 1)],
                                              wps[:], 8.0)
                            dk = dpool.tile([128, WIN], fp8, tag="dk", name="dk")
                            nc.sync.dma_start(dk[:], w8k[:])
                            kd_dram[hh].append(dk)

                    # -- per head: scores, softmax, ctx (pc lags one t), out --
                    for hh in range(2):
                        h = 2 * P + hh
                        rs = slice(64 * hh, 64 * (hh + 1))
                        s1 = s1tiles[hh]

                        pc = ctxps.tile([65, 1024], f32, tag="ctx", name="ctx")

                        def emit_pc(t, prh):
                            for half in range(2):
                                sl = slice(512 * half, 512 * (half + 1))
                                nc.tensor.matmul(
                                    pc[:, sl], vh[t][:, 65 * h:65 * (h + 1)],
                                    prh[half][:],
                                    start=(t == 0), stop=(t == NT - 1),
                                    skip_group_check=True)

                        prev = None
                        for t in range(NT):
                            s2 = sblkpool.tile([128, 1024], fp8, tag="s2", name="s2")
                            nc.sync.dma_start(s2[:], kd_dram[hh][t][:].__replace__(
                                ap=[[1151, 128], [1, 1024]], offset=127))

                            prhalf = []
                            for half in range(2):
                                sl = slice(512 * half, 512 * (half + 1))
                                sc = scps.tile([128, 512], f32, tag="sc", name="sc")
                                for Lh in range(4):
                                    L = 4 * half + Lh
                                    nc.tensor.matmul(
                                        sc[:, 128 * Lh:128 * (Lh + 1)],
                                        s1[L][:, 128 * t:128 * (t + 1)], id8[:],
                                        start=(Lh == 0), stop=False,
                                        skip_group_check=True)
                                nc.tensor.matmul(sc[:],
                                                 kT[rs, 128 * t:128 * (t + 1)],
                                                 qT[rs, sl],
                                                 start=False, stop=False,
                                                 skip_group_check=True)
                                nc.tensor.matmul(sc[:], id8[:], s2[:, sl],
                                                 start=False, stop=True,
                                                 skip_group_check=True)
                                pr = prpool.tile([128, 512], bf16, tag="pr", name="pr")
                                prhalf.append(pr)
                                nc.scalar.activation(pr[:], sc[:], Exp,
                                                     bias=mhat_sb[:, t:t + 1],
                                                     scale=0.125)
                            if prev is not None:
                                emit_pc(t - 1, prev)
                            prev = prhalf
                        emit_pc(NT - 1, prev)

                        cs = ctxsb.tile([65, 1024], bf16, tag="cs", name="cs")
                        nc.scalar.copy(cs[:], pc[:])
                        for L in range(NT):
                            po = winps.tile([128, 65], bf16, tag="win", name="po")
                            nc.tensor.transpose(
                                po[:], cs[:, 128 * L:128 * (L + 1)],
                                idb[0:65, 0:65])
                            rec = osmall.tile([128, 1], f32, tag="rec", name="rec")
                            nc.vector.reciprocal(rec[:], po[:, 64:65])
                            nc.vector.tensor_scalar_mul(
                                outacc[L][:, 64 * h:64 * (h + 1)],
                                po[:, 0:64], rec[:])

            for L in range(NT):
                nc.sync.dma_start(out_d.ap()[128 * L:128 * (L + 1), :], outacc[L][:])

    nc.compile()
    return nc


def run_cores(nc, in_maps, core_ids=None, trace=False):
    from concourse.bass_utils import run_bass_kernel_spmd
    if core_ids is None:
        core_ids = list(range(len(in_maps)))
    return run_bass_kernel_spmd(nc, in_maps, core_ids, trace=trace)


_NC_CACHE = {}
_LAST = {"exec_time_ns": None}


def _get_program():
    if "nc" not in _NC_CACHE:
        _NC_CACHE["nc"] = build_program()
    return _NC_CACHE["nc"]


def get_last_exec_time_ns():
    return _LAST["exec_time_ns"]


def kernel(hidden_states, attention_mask, W_qkv, b_qkv, dist_emb):
    from concourse.bass_utils import run_bass_kernel_spmd

    hidden_states = np.asarray(hidden_states, dtype=np.float32)
    attention_mask = np.asarray(attention_mask, dtype=np.float32)
    W_qkv = np.asarray(W_qkv, dtype=np.float32)
    b_qkv = np.asarray(b_qkv, dtype=np.float32)
    dist_emb = np.asarray(dist_emb, dtype=np.float32)

    B = hidden_states.shape[0]
    nc = _get_program()
    in_maps = host_prep(hidden_states, attention_mask, W_qkv, b_qkv, dist_emb)
    trace = bool(os.environ.get("BASS_TRACE"))
    res = run_bass_kernel_spmd(nc, in_maps, list(range(B)), trace=trace)
    _LAST["exec_time_ns"] = res.exec_time_ns
    out = np.stack([res.results[i]["out"] for i in range(B)], axis=0)
    return out.astype(np.float32)


# revision 37
# speedup vs baseline: 1.0157x; 1.0157x over previous
"""Self-contained Trainium2 Bass kernel for BertSelfAttention (relative_key_query).

kernel(**inputs) takes FULL unsharded inputs (as in setup_inputs()) and returns
the FULL (8, 1024, 1024) float32 output. Internally: data-parallel over the
batch dimension, one batch per NeuronCore across 8 cores, via
concourse run_bass_kernel_spmd.

Math (per batch b): qkv = hs @ W_qkv + b; per-head q,k,v (dh=64);
scores = (q k^T + q.pe^T-window + k.pe-window)/8 + mask; softmax; ctx = p v.
The relative-position terms are computed as banded "windows" against the
2047x64 distance table, stored to DRAM in fp8 (scaled by 8), and read back
with a skewed access pattern that materializes the diagonal bands densely.
Dense matmuls run in bf16; window/score-injection matmuls run in fp8.
"""

import os
import numpy as np


import concourse.bacc as bacc
import concourse.mybir as mybir
import concourse.tile as tile

f32 = mybir.dt.float32
bf16 = mybir.dt.bfloat16
fp8 = mybir.dt.float8e4

S = 1024
D = 1024
H = 16
DH = 64
NT = 8
WIN = 1152
NPAIR = 8


def host_prep(hidden_states, attention_mask, W_qkv, b_qkv, dist_emb):
    import ml_dtypes
    bfl = ml_dtypes.bfloat16
    e4 = ml_dtypes.float8_e4m3fn

    B = hidden_states.shape[0]
    W = np.asarray(W_qkv, dtype=np.float32)
    b = np.asarray(b_qkv, dtype=np.float32)
    T = np.asarray(dist_emb, dtype=np.float32)

    qcols = np.zeros((8, 128), dtype=np.int64)
    kcols = np.zeros((8, 128), dtype=np.int64)
    for t in range(8):
        for j in range(128):
            h = 2 * t + (j >= 64)
            d = j % 64
            qcols[t, j] = h * 192 + d
            kcols[t, j] = h * 192 + 64 + d
    qk_idx = np.concatenate([qcols.reshape(-1), kcols.reshape(-1)])
    WQK = np.ascontiguousarray(W[:, qk_idx]).astype(bfl)
    bQK = np.ascontiguousarray(b[qk_idx].reshape(16, 128).T)
    vidx = np.array([h * 192 + 128 + d for h in range(H) for d in range(DH)])
    WV = np.ascontiguousarray(W[:, vidx]).astype(bfl)
    bV = np.ascontiguousarray(b[vidx].reshape(1, 1024)).astype(bfl)

    T2 = np.zeros((128, 2048), dtype=np.float32)
    T2[0:64, 0:2047] = T.T
    T2[64:128, 0:2047] = T.T
    T2R = np.zeros((128, 2048), dtype=np.float32)
    T2R[0:64, 0:2047] = T.T[:, ::-1]
    T2R[64:128, 0:2047] = T.T[:, ::-1]
    T2 = T2.astype(bfl)
    T2R = T2R.astype(bfl)

    ones_r = np.ones((1, 128), dtype=np.float32).astype(bfl)
    # identity used to inject fp8 windows into the score PSUM; its value
    # 0.125 cancels the x8 scaling applied when the windows were stored.
    id8_h = (0.125 * np.eye(128, dtype=np.float32)).astype(e4)
    idb_h = np.eye(128, dtype=np.float32).astype(bfl)
    idf_h = np.eye(128, dtype=np.float32)

    mask = np.asarray(attention_mask, dtype=np.float32).reshape(B, S)
    in_maps = []
    for bi in range(B):
        mhat = np.ascontiguousarray(mask[bi].reshape(8, 128).T)
        in_maps.append({
            "hs": np.ascontiguousarray(hidden_states[bi], dtype=np.float32),
            "wqk": WQK, "bqk": bQK, "wv": WV, "bv": bV,
            "t2": T2, "t2r": T2R, "ones_r": ones_r, "mhat": mhat,
            "id8_h": id8_h, "idb_h": idb_h, "idf_h": idf_h,
        })
    return in_maps


def build_program(npair=NPAIR):
    nc = bacc.Bacc()
    hs_d = nc.declare_dram_parameter("hs", [S, D], f32, isOutput=False)
    wqk_d = nc.declare_dram_parameter("wqk", [D, 2048], bf16, isOutput=False)
    bqk_d = nc.declare_dram_parameter("bqk", [128, 16], f32, isOutput=False)
    wv_d = nc.declare_dram_parameter("wv", [D, 1024], bf16, isOutput=False)
    bv_d = nc.declare_dram_parameter("bv", [1, 1024], bf16, isOutput=False)
    t2_d = nc.declare_dram_parameter("t2", [128, 2048], bf16, isOutput=False)
    t2r_d = nc.declare_dram_parameter("t2r", [128, 2048], bf16, isOutput=False)
    ones_d = nc.declare_dram_parameter("ones_r", [1, 128], bf16, isOutput=False)
    mhat_d = nc.declare_dram_parameter("mhat", [128, 8], f32, isOutput=False)
    id8_d = nc.declare_dram_parameter("id8_h", [128, 128], fp8, isOutput=False)
    idb_d = nc.declare_dram_parameter("idb_h", [128, 128], bf16, isOutput=False)
    idf_d = nc.declare_dram_parameter("idf_h", [128, 128], f32, isOutput=False)
    out_d = nc.declare_dram_parameter("out", [S, D], f32, isOutput=True)

    Exp = mybir.ActivationFunctionType.Exp
    Ident = mybir.ActivationFunctionType.Identity

    with tile.TileContext(nc) as tc:
        with tc.tile_pool(name="const", bufs=1) as cpool:
            t2_sb = cpool.tile([128, 2048], bf16, tag="t2", name="t2")
            t2r_sb = cpool.tile([128, 2048], bf16, tag="t2r", name="t2r")
            bqk_sb = cpool.tile([128, 16], f32, tag="bqk", name="bqk")
            nc.sync.dma_start(bqk_sb[:], bqk_d.ap())
            bv_sb = cpool.tile([1, 1024], bf16, tag="bv", name="bv")
            nc.sync.dma_start(bv_sb[:], bv_d.ap())
            ones_sb = cpool.tile([1, 128], bf16, tag="ones", name="ones")
            nc.sync.dma_start(ones_sb[:], ones_d.ap())
            mhat_sb = cpool.tile([128, 8], f32, tag="mh", name="mh")
            nc.sync.dma_start(mhat_sb[:], mhat_d.ap())
            id8 = cpool.tile([128, 128], fp8, tag="id8", name="id8")
            nc.sync.dma_start(id8[:], id8_d.ap())
            idb = cpool.tile([128, 128], bf16, tag="idb", name="idb")
            nc.sync.dma_start(idb[:], idb_d.ap())
            idf = cpool.tile([128, 128], f32, tag="idf", name="idf")
            nc.sync.dma_start(idf[:], idf_d.ap())
            ones16 = cpool.tile([128, 16], bf16, tag="o16", name="o16")
            nc.vector.memset(ones16[:], 1.0)

            # resident weights (bf16); DMAs are emitted after phase 1 so the
            # hs loads that gate the first transposes go out first.
            wqk_sb = [cpool.tile([128, 2048], bf16, tag=f"wqk{i}", name=f"wqk{i}")
                      for i in range(NT)]
            wv_sb = [cpool.tile([128, 1024], bf16, tag=f"wv{i}", name=f"wv{i}")
                     for i in range(NT)]

            hsT = [cpool.tile([128, 1024], bf16, tag=f"hsT{i}", name=f"hsT{i}") for i in range(NT)]
            vh = [cpool.tile([128, 1040], bf16, tag=f"vh{t}", name=f"vh{t}") for t in range(NT)]
            outacc = [cpool.tile([128, 1024], f32, tag=f"oa{i}", name=f"oa{i}") for i in range(NT)]
            for L in range(NT):
                nc.gpsimd.memset(outacc[L][:], 0.0)

            # ---- Phase 1: hs -> bf16 -> hsT via PE transpose ----
            with tc.tile_pool(name="p1", bufs=2) as p1, \
                 tc.tile_pool(name="p1ps", bufs=2, space="PSUM") as p1ps:
                for lt in range(NT):
                    hstile = p1.tile([128, 1024], f32, tag="hs", name="hs")
                    nc.sync.dma_start(hstile[:], hs_d.ap()[128 * lt:128 * (lt + 1), :])
                    hsb = p1.tile([128, 1024], bf16, tag="hsb", name="hsb")
                    nc.gpsimd.tensor_copy(hsb[:], hstile[:])
                    for ig in range(2):
                        ps = p1ps.tile([128, 512], bf16, tag="tp", name="tp")
                        for j in range(4):
                            it = 4 * ig + j
                            nc.tensor.matmul(
                                ps[:, 128 * j:128 * (j + 1)],
                                hsb[:, 128 * it:128 * (it + 1)], idb[:],
                                is_transpose=True,
                                start=(j == 0), stop=(j == 3),
                                skip_group_check=True)
                        for j in range(4):
                            it = 4 * ig + j
                            nc.scalar.copy(
                                hsT[it][:, 128 * lt:128 * (lt + 1)],
                                ps[:, 128 * j:128 * (j + 1)])
                    if lt == 0:
                        for i in range(NT):
                            nc.sync.dma_start(
                                wv_sb[i][:], wv_d.ap()[128 * i:128 * (i + 1), :])
                    if lt == 1:
                        for i in range(NT):
                            nc.sync.dma_start(
                                wqk_sb[i][:],
                                wqk_d.ap()[128 * i:128 * (i + 1), :])
                    if lt == 2:
                        nc.sync.dma_start(t2_sb[:], t2_d.ap())
                        nc.sync.dma_start(t2r_sb[:], t2r_d.ap())

            # ---- Phase 2: v-hat tiles (+ones col, +bias) ----
            with tc.tile_pool(name="p2ps", bufs=3, space="PSUM") as p2ps:
                for tau in range(NT):
                    lt = tau
                    psv = p2ps.tile([128, 1024], f32, tag="vps", name="vps")
                    for half in range(2):
                        sl = slice(512 * half, 512 * (half + 1))
                        nc.tensor.matmul(psv[:, sl], ones_sb[:], bv_sb[:, sl],
                                         start=True, stop=False,
                                         skip_group_check=True)
                    for it in range(NT):
                        for half in range(2):
                            sl = slice(512 * half, 512 * (half + 1))
                            nc.tensor.matmul(psv[:, sl],
                                             hsT[it][:, 128 * lt:128 * (lt + 1)],
                                             wv_sb[it][:, sl],
                                             start=False, stop=(it == NT - 1),
                                             skip_group_check=True)
                    out_ap = vh[tau][:].__replace__(
                        ap=[[1040, 128], [65, 16], [1, 64]], offset=0)
                    in_ap = psv[:].__replace__(
                        ap=[[1024, 128], [64, 16], [1, 64]], offset=0)
                    nc.scalar.copy(out_ap, in_ap)
                    ones_ap = vh[tau][:].__replace__(
                        ap=[[1040, 128], [65, 16]], offset=64)
                    nc.scalar.copy(ones_ap, ones16[:])

            # ---- Phase 3: per head-pair ----
            with tc.tile_pool(name="qk", bufs=2) as qkpool, \
                 tc.tile_pool(name="winsb", bufs=6) as winsb, \
                 tc.tile_pool(name="sblk", bufs=6) as sblkpool, \
                 tc.tile_pool(name="s1blk", bufs=24) as s1pool, \
                 tc.tile_pool(name="probs", bufs=6) as prpool, \
                 tc.tile_pool(name="ctxsb", bufs=2) as ctxsb, \
                 tc.tile_pool(name="osmall", bufs=4) as osmall, \
                 tc.tile_pool(name="dram", bufs=36, space="DRAM") as dpool, \
                 tc.tile_pool(name="small", bufs=1, space="PSUM") as smallps, \
                 tc.tile_pool(name="winps", bufs=3, space="PSUM") as winps, \
                 tc.tile_pool(name="scps", bufs=2, space="PSUM") as scps, \
                 tc.tile_pool(name="ctxps", bufs=1, space="PSUM") as ctxps:
                for P in range(npair):
                    # -- 3a: q^T, k^T (bf16) --
                    qT = qkpool.tile([128, 1024], bf16, tag="qT", name="qT")
                    kT = qkpool.tile([128, 1024], bf16, tag="kT", name="kT")
                    for dst, ct in ((qT, P), (kT, 8 + P)):
                        for half in range(2):
                            sl = slice(512 * half, 512 * (half + 1))
                            ps = smallps.tile([128, 512], f32, tag="small", name="ps")
                            for it in range(NT):
                                nc.tensor.matmul(
                                    ps[:],
                                    wqk_sb[it][:, 128 * ct:128 * (ct + 1)],
                                    hsT[it][:, sl],
                                    start=(it == 0), stop=(it == NT - 1),
                                    skip_group_check=True)
                            nc.scalar.activation(dst[:, sl], ps[:], Ident,
                                                 bias=bqk_sb[:, ct:ct + 1],
                                                 scale=1.0)

                    # -- 3b: windows -> fp8 (x8) -> DRAM, plus skewed reads --
                    # casts are spread over vector/gpsimd/scalar so the PE
                    # window matmuls are not cast-throughput-bound.
                    qd_dram = [[], []]
                    kd_dram = [[], []]
                    s1tiles = [[], []]
                    for hh in range(2):
                        rs = slice(64 * hh, 64 * (hh + 1))
                        for t in range(NT):
                            base = 896 - 128 * t
                            # q-side: fp8 (x8) — feeds the weight-load-bound
                            # transpose injects, which need 1-byte weights.
                            w8q = winsb.tile([128, WIN], fp8, tag="w8", name="w8")
                            for c3 in range(3):
                                wps = winps.tile([128, 384], f32, tag="win", name="wps")
                                nc.tensor.matmul(
                                    wps[:], qT[rs, 128 * t:128 * (t + 1)],
                                    t2r_sb[rs, base + 384 * c3:base + 384 * (c3 + 1)],
                                    start=True, stop=True, skip_group_check=True)
                                nc.vector.tensor_scalar_mul(
                                    w8q[:, 384 * c3:384 * (c3 + 1)], wps[:], 8.0)

                            dq = dpool.tile([128, WIN], fp8, tag="dq", name="dq")
                            nc.sync.dma_start(dq[:], w8q[:])
                            qd_dram[hh].append(dq)
                            blk = s1pool.tile([128, 1024], fp8, tag="s1", name="s1")
                            nc.sync.dma_start(blk[:], dq[:].__replace__(
                                ap=[[1151, 128], [1, 1024]], offset=127))
                            s1tiles[hh].append(blk)

                            # k-side: fp8 (x8), injected as moving data
                            w8k = winsb.tile([128, WIN], fp8, tag="w8k", name="w8k")
                            for c3 in range(3):
                                wps = winps.tile([128, 384], f32, tag="win", name="wps")
                                nc.tensor.matmul(
                                    wps[:], kT[rs, 128 * t:128 * (t + 1)],
                                    t2_sb[rs, base + 384 * c3:base + 384 * (c3 + 1)],
                                    start=True, stop=True, skip_group_check=True)
                                nc.scalar.mul(w8k[:, 384 * c3:384 * (c3 + 1)],
                                              wps[:], 8.0)
                            dk = dpool.tile([128, WIN], fp8, tag="dk", name="dk")
                            nc.sync.dma_start(dk[:], w8k[:])
                            kd_dram[hh].append(dk)

                    # -- per head: scores, softmax, ctx (pc lags one t), out --
                    for hh in range(2):
                        h = 2 * P + hh
                        rs = slice(64 * hh, 64 * (hh + 1))
                        s1 = s1tiles[hh]

                        pc = ctxps.tile([65, 1024], f32, tag="ctx", name="ctx")

                        def emit_pc(t, prh):
                            for half in range(2):
                                sl = slice(512 * half, 512 * (half + 1))
                                nc.tensor.matmul(
                                    pc[:, sl], vh[t][:, 65 * h:65 * (h + 1)],
                                    prh[half][:],
                                    start=(t == 0), stop=(t == NT - 1),
                                    skip_group_check=True)

                        prev = None
                        for t in range(NT):
                            s2 = sblkpool.tile([128, 1024], fp8, tag="s2", name="s2")
                            nc.sync.dma_start(s2[:], kd_dram[hh][t][:].__replace__(
                                ap=[[1151, 128], [1, 1024]], offset=127))

                            prhalf = []
                            for half in range(2):
                                sl = slice(512 * half, 512 * (half + 1))
                                sc = scps.tile([128, 512], f32, tag="sc", name="sc")
                                for Lh in range(4):
                                    L = 4 * half + Lh
                                    nc.tensor.matmul(
                                        sc[:, 128 * Lh:128 * (Lh + 1)],
                                        s1[L][:, 128 * t:128 * (t + 1)], id8[:],
                                        start=(Lh == 0), stop=False,
                                        skip_group_check=True)
                                nc.tensor.matmul(sc[:],
                                                 kT[rs, 128 * t:128 * (t + 1)],
                                                 qT[rs, sl],
                                                 start=False, stop=False,
                                                 skip_group_check=True)
                                nc.tensor.matmul(sc[:], id8[:], s2[:, sl],
                                                 start=False, stop=True,
                                                 skip_group_check=True)
                                pr = prpool.tile([128, 512], bf16, tag="pr", name="pr")
                                prhalf.append(pr)
                                nc.scalar.activation(pr[:], sc[:], Exp,
                                                     bias=mhat_sb[:, t:t + 1],
                                                     scale=0.125)
                            if prev is not None:
                                emit_pc(t - 1, prev)
                            prev = prhalf
                        emit_pc(NT - 1, prev)

                        cs = ctxsb.tile([65, 1024], bf16, tag="cs", name="cs")
                        nc.scalar.copy(cs[:], pc[:])
                        for L in range(NT):
                            po = winps.tile([128, 65], bf16, tag="win", name="po")
                            nc.tensor.transpose(
                                po[:], cs[:, 128 * L:128 * (L + 1)],
                                idb[0:65, 0:65])
                            rec = osmall.tile([128, 1], f32, tag="rec", name="rec")
                            nc.vector.reciprocal(rec[:], po[:, 64:65])
                            nc.vector.tensor_scalar_mul(
                                outacc[L][:, 64 * h:64 * (h + 1)],
                                po[:, 0:64], rec[:])

            for L in range(NT):
                nc.sync.dma_start(out_d.ap()[128 * L:128 * (L + 1), :], outacc[L][:])

    nc.compile()
    return nc


def run_cores(nc, in_maps, core_ids=None, trace=False):
    from concourse.bass_utils import run_bass_kernel_spmd
    if core_ids is None:
        core_ids = list(range(len(in_maps)))
    return run_bass_kernel_spmd(nc, in_maps, core_ids, trace=trace)


_NC_CACHE = {}
_LAST = {"exec_time_ns": None}


def _get_program():
    if "nc" not in _NC_CACHE:
        _NC_CACHE["nc"] = build_program()
    return _NC_CACHE["nc"]


def get_last_exec_time_ns():
    return _LAST["exec_time_ns"]


def kernel(hidden_states, attention_mask, W_qkv, b_qkv, dist_emb):
    from concourse.bass_utils import run_bass_kernel_spmd

    hidden_states = np.asarray(hidden_states, dtype=np.float32)
    attention_mask = np.asarray(attention_mask, dtype=np.float32)
    W_qkv = np.asarray(W_qkv, dtype=np.float32)
    b_qkv = np.asarray(b_qkv, dtype=np.float32)
    dist_emb = np.asarray(dist_emb, dtype=np.float32)

    B = hidden_states.shape[0]
    nc = _get_program()
    in_maps = host_prep(hidden_states, attention_mask, W_qkv, b_qkv, dist_emb)
    trace = bool(os.environ.get("BASS_TRACE"))
    res = run_bass_kernel_spmd(nc, in_maps, list(range(B)), trace=trace)
    _LAST["exec_time_ns"] = res.exec_time_ns
    out = np.stack([res.results[i]["out"] for i in range(B)], axis=0)
    return out.astype(np.float32)
